# revision 33
# baseline (speedup 1.0000x reference)
"""Trainium2 Bass kernel for nn_AttentionBlock (column-softmax causal attention).

Reference computation (B=4, S=4096, D=128, K=64, V=128):
    Q = x @ Wq.T + bq            [B,S,64]
    Km = x @ Wk.T + bk           [B,S,64]
    Vm = x @ Wv.T + bv           [B,S,128]
    s  = Q @ Km.T / 8            [B,S,S], causal mask j>q -> -1e9
    p  = softmax(s, axis=1)      (softmax over the QUERY axis -- column softmax)
    att = p @ Vm                 [B,S,128]
    out = concat(x, att, dim=2)  [B,S,256]

Algebraic restructure (lets every matmul run fp16 at 1 cyc/row with full
128-deep contraction):
    s[q,j] = x_q M x_j^T + x_q.a + x_j.b + c   with M = Wq^T Wk / 8,
             a = Wq^T bk / 8, b = Wk^T bq / 8, c = bq.bk / 8.
    The (x_j.b + c) term is constant along the softmax (q) axis for a fixed
    column j, so it CANCELS in softmax(dim=q) and is dropped entirely.
    The x_q.a term folds into G: with G_j = M x_j^T + a (per-partition add),
    s^T[j,q] = sum_d G[d,j] * xT[d,q].
    So: GT = M @ xkv^T + a (tiny matmul), then scores are a single fp16
    128-contraction matmul per (j-tile, q-chunk). No Q/K projections at all.

Flash-style column softmax as in the baseline: E[j,q] = exp(s^T), masked
entries are exp(-1e9)=0; l[j] = sum_q E[j,q] (free-dim reduce);
att^T[v,q] = sum_j (V[j,v]/l[j]) * E[j,q]. Output stays in [v,q] layout --
the HOST transposes (no PE transposes anywhere on device).

Sharding (8 cores): core c -> batch b = c//2, j-tile parity p = c%2.
Host adds the two partial att's per batch.

Pipeline: forward row order (row i = local j-tile i), PV chunk c emitted
after row 2c+1, so the PV matmuls run interleaved with later rows' score
matmuls and the exp stream -- no serial phase C.
"""

import numpy as np

B, S, D = 4, 4096, 128
KD, VD = 64, 128
P = 128
NCORES = 8
JT = 16           # local j-tiles per core
CHUNK = 1536      # score chunk width (PSUM cols, 3 banks)

ROW_W = [S - 2 * i * P for i in range(JT)]          # E row widths
EOFF = [0] * JT
for _i in range(1, JT):
    EOFF[_i] = EOFF[_i - 1] + ROW_W[_i - 1]
ECOLS = EOFF[-1] + ROW_W[-1]                        # 34816

_CACHE = {}


def _build_program():
    from contextlib import ExitStack

    from concourse import bacc, mybir
    from concourse import tile as tile_mod

    dt = mybir.dt
    f32, bf16 = dt.float32, dt.bfloat16
    Alu = mybir.AluOpType
    ActF = mybir.ActivationFunctionType

    nc = bacc.Bacc(
        "TRN2", target_bir_lowering=False, debug=False, num_devices=NCORES
    )

    xt_d = nc.dram_tensor("xt", [P, S], bf16, kind="ExternalInput").ap()
    xkvt_d = nc.dram_tensor("xkvt", [P, JT * P], bf16, kind="ExternalInput").ap()
    # s16: Mt[0:128] | WvT[128:256]
    s16_d = nc.dram_tensor("s16", [P, 256], bf16, kind="ExternalInput").ap()
    # s32: a[0] | bvb[1:129]   s32m: mrow
    s32_d = nc.dram_tensor("s32", [P, 129], f32, kind="ExternalInput").ap()
    s32m_d = nc.dram_tensor("s32m", [P, 256], f32, kind="ExternalInput").ap()
    att_d = nc.dram_tensor("att", [P, S], f32, kind="ExternalOutput").ap()

    with tile_mod.TileContext(nc) as tc, ExitStack() as ctx:
        persist = ctx.enter_context(tc.tile_pool(name="persist", bufs=1))

        xT = persist.tile([P, S], bf16)            # [d, q]
        xkvT = persist.tile([P, JT * P], bf16)     # [d, local j]
        GT = persist.tile([P, JT * P], bf16)       # [d, local j] = M xkv^T + a
        E_all = persist.tile([P, ECOLS], bf16)     # exp(scores^T) rows
        Vp = persist.tile([P, JT, VD], bf16)       # [j, v] scaled by 1/l
        l_all = persist.tile([P, JT], f32)
        linv = persist.tile([P, JT], f32)
        lp2 = persist.tile([P, JT], f32)           # chunk-1 l partials
        lp3 = persist.tile([P, JT], f32)           # chunk-2 l partials
        V_sb = persist.tile([P, JT, VD], f32)      # V + bv, unscaled
        o5a = persist.tile([P, 512], f32)          # PV chunk-5 early partial
        o6a = persist.tile([P, 512], f32)          # PV chunk-6 early partial
        o7a = persist.tile([P, 512], f32)          # PV chunk-7 early partial
        warm = persist.tile([P, 8], f32)           # exp-table warmup scratch
        s16 = persist.tile([P, 256], bf16)
        s32 = persist.tile([P, 129], f32)
        s32m = persist.tile([P, 256], f32)
        Mt = s16[:, 0:128]
        WvT = s16[:, 128:256]
        a_sb = s32[:, 0:1]
        bvb = s32[:, 1:129]
        mrow = s32m

        # ---- input DMAs: critical pieces ride the HWDGE queues (sync/
        # scalar); the SWDGE gpsimd queue gets the small/late pieces
        nc.sync.dma_start(out=s16, in_=s16_d)
        nc.sync.dma_start(out=xkvT[:, 0:512], in_=xkvt_d[:, 0:512])
        nc.sync.dma_start(out=xkvT[:, 1024:2048], in_=xkvt_d[:, 1024:2048])
        nc.sync.dma_start(out=xT[:, 3072:4096], in_=xt_d[:, 3072:4096])
        nc.scalar.dma_start(out=xT[:, 0:1024], in_=xt_d[:, 0:1024])
        nc.scalar.dma_start(out=s32, in_=s32_d)
        nc.scalar.dma_start(out=xT[:, 1024:2048], in_=xt_d[:, 1024:2048])
        nc.scalar.dma_start(out=xT[:, 2048:3072], in_=xt_d[:, 2048:3072])
        nc.gpsimd.dma_start(out=s32m, in_=s32m_d)
        nc.gpsimd.dma_start(out=xkvT[:, 512:1024], in_=xkvt_d[:, 512:1024])

        # load the EXP activation table while DMAs land
        nc.gpsimd.memset(warm, 0.0)
        nc.scalar.activation(
            out=warm, in_=warm, func=ActF.Exp
        )

        with ExitStack() as ph:
            ps = ph.enter_context(
                tc.tile_pool(name="ps", bufs=2, space="PSUM")
            )
            aux = ph.enter_context(
                tc.tile_pool(name="aux", bufs=2, space="PSUM")
            )
            osb = ph.enter_context(tc.tile_pool(name="osb", bufs=2))

            def emit_gt(g):
                pgt = aux.tile([P, 512], f32, tag="aux", name=f"gt_{g}")
                nc.tensor.matmul(
                    pgt,
                    lhsT=Mt,
                    rhs=xkvT[:, g * 512 : (g + 1) * 512],
                    start=True,
                    stop=True,
                )
                nc.vector.tensor_scalar(
                    out=GT[:, g * 512 : (g + 1) * 512],
                    in0=pgt,
                    scalar1=a_sb,
                    scalar2=None,
                    op0=Alu.add,
                )

            def emit_qk_row(i):
                w = ROW_W[i]
                q0 = 256 * i
                # row 0's first chunk needs only the first xT quarter, so
                # the exp stream starts as soon as that DMA lands
                if i == 0:
                    bounds = [0, 1024, 2560, 4096]
                else:
                    bounds = list(range(0, w, CHUNK)) + [w]
                nch = len(bounds) - 1
                for ci in range(nch):
                    c0, cw = bounds[ci], bounds[ci + 1] - bounds[ci]
                    sc = ps.tile([P, CHUNK], f32, tag="ps", name=f"sc_{i}_{ci}")
                    for s0 in range(0, cw, 512):
                        sw = min(512, cw - s0)
                        off = q0 + c0 + s0
                        nc.tensor.matmul(
                            sc[:, s0 : s0 + sw],
                            lhsT=GT[:, i * P : (i + 1) * P],
                            rhs=xT[:, off : off + sw],
                            start=True,
                            stop=True,
                        )
                        if ci == 0 and s0 == 0:
                            # mask the diagonal block as soon as its slice
                            # lands (overlaps the remaining slice matmuls)
                            nc.vector.tensor_tensor(
                                out=sc[:, : 2 * P],
                                in0=sc[:, : 2 * P],
                                in1=mrow,
                                op=Alu.add,
                            )
                    ecol = EOFF[i] + c0
                    nc.scalar.activation(
                        out=E_all[:, ecol : ecol + cw],
                        in_=sc[:, :cw],
                        func=ActF.Exp,
                        accum_out=(
                            l_all[:, i : i + 1] if ci == 0 else None
                        ),
                    )
                if nch > 1:
                    # l tail from one DVE reduce over the row's remaining E
                    # (bf16, contiguous) -- keeps the reads off the ACT
                    nc.vector.tensor_reduce(
                        out=lp2[:, i : i + 1],
                        in_=E_all[:, EOFF[i] + bounds[1] : EOFF[i] + w],
                        axis=mybir.AxisListType.X,
                        op=Alu.add,
                    )
                    nc.vector.tensor_tensor(
                        out=l_all[:, i : i + 1],
                        in0=l_all[:, i : i + 1],
                        in1=lp2[:, i : i + 1],
                        op=Alu.add,
                    )
                nc.vector.reciprocal(linv[:, i : i + 1], l_all[:, i : i + 1])
                nc.vector.tensor_scalar(
                    out=Vp[:, i, :],
                    in0=V_sb[:, i, :],
                    scalar1=linv[:, i : i + 1],
                    scalar2=None,
                    op0=Alu.mult,
                )

            def emit_v(i):
                # V projection for tile i (runs at startup; needs no l)
                pv = aux.tile([P, VD], f32, tag="aux", name=f"v_{i}")
                nc.tensor.matmul(
                    pv,
                    lhsT=xkvT[:, i * P : (i + 1) * P],
                    rhs=WvT,
                    start=True,
                    stop=True,
                )
                nc.vector.tensor_tensor(
                    out=V_sb[:, i, :], in0=pv, in1=bvb, op=Alu.add
                )

            def emit_pv(c, lo=0, hi=None, merge=None, out_sb=None):
                # att^T chunk c over full-width rows [lo, hi); when hi is
                # None also the half-coverage row 2c+1 closes the group
                tail = hi is None
                hi2 = 2 * c + 1 if tail else hi
                ap = aux.tile([P, 512], f32, tag="aux", name=f"att_{c}_{lo}")
                for ii in range(lo, hi2):
                    ecol = EOFF[ii] + 512 * c - 256 * ii
                    nc.tensor.matmul(
                        ap,
                        lhsT=Vp[:, ii, :],
                        rhs=E_all[:, ecol : ecol + 512],
                        start=(ii == lo),
                        stop=(not tail and ii == hi2 - 1),
                    )
                if tail:
                    i2 = 2 * c + 1
                    nc.tensor.matmul(
                        ap[:, 256:512],
                        lhsT=Vp[:, i2, :],
                        rhs=E_all[:, EOFF[i2] : EOFF[i2] + 256],
                        start=False,
                        stop=True,
                    )
                if out_sb is not None:
                    nc.vector.tensor_copy(out_sb, ap)
                    return
                ob = osb.tile([P, 512], f32, tag="osb", name=f"osb_{c}")
                if merge is None:
                    nc.vector.tensor_copy(ob, ap)
                else:
                    nc.vector.tensor_tensor(
                        out=ob, in0=ap, in1=merge, op=Alu.add
                    )
                nc.sync.dma_start(
                    out=att_d[:, c * 512 : (c + 1) * 512], in_=ob
                )

            emit_gt(0)
            emit_v(0)
            emit_v(1)
            for i in range(JT):
                if i == 1:
                    for t in range(2, 4):
                        emit_v(t)
                    emit_gt(1)
                    for t in range(4, 8):
                        emit_v(t)
                    emit_gt(2)
                    for t in range(8, 12):
                        emit_v(t)
                if i == 2:
                    emit_gt(3)
                    for t in range(12, 16):
                        emit_v(t)
                emit_qk_row(i)
                # PV schedule: early partials of the late chunks run as soon
                # as their rows' E lands, so the post-exp tail stays tiny
                if i % 2 == 1 and i <= 9:
                    emit_pv((i - 1) // 2)
                if i == 8:
                    emit_pv(5, lo=0, hi=8, out_sb=o5a)
                if i == 10:
                    emit_pv(6, lo=0, hi=10, out_sb=o6a)
                if i == 11:
                    emit_pv(5, lo=8, merge=o5a)
                    emit_pv(7, lo=0, hi=12, out_sb=o7a)
                if i == 13:
                    emit_pv(6, lo=10, merge=o6a)
            emit_pv(7, lo=12, merge=o7a)

    nc.compile()
    return nc


def _host_inputs(x, Wq, bq, Wk, bk, Wv, bv):
    """Per-core input maps (host does layout prep + tiny precomputes)."""
    import ml_dtypes

    hf = ml_dtypes.bfloat16
    x_full = np.ascontiguousarray(x, dtype=np.float32)
    Wq = np.asarray(Wq, np.float32)
    Wk = np.asarray(Wk, np.float32)
    bk = np.asarray(bk, np.float32)
    Wv = np.asarray(Wv, np.float32)
    bv = np.asarray(bv, np.float32)

    M = (Wq.T @ Wk) / 8.0                      # [D, D]
    Mt = np.ascontiguousarray(M.T).astype(hf)
    a = ((Wq.T @ bk) / 8.0).reshape(D, 1)      # [D, 1]
    WvT = np.ascontiguousarray(Wv.T).astype(hf)
    bvb = np.tile(bv.reshape(1, VD), (P, 1))   # [P, V]

    tri = np.where(
        np.arange(P)[None, :] >= np.arange(P)[:, None], 0.0, -1e9
    ).astype(np.float32)
    mrows = []
    for p in (0, 1):
        m = np.zeros((P, 2 * P), np.float32)
        if p == 0:
            m[:, :P] = tri
        else:
            m[:, :P] = -1e9
            m[:, P:] = tri
        mrows.append(m)

    s16 = np.ascontiguousarray(np.concatenate([Mt, WvT], axis=1))
    s32 = np.ascontiguousarray(
        np.concatenate([a, bvb], axis=1).astype(np.float32)
    )
    xts = [
        np.ascontiguousarray(x_full[b].T.astype(hf)) for b in range(B)
    ]
    in_maps = []
    for c in range(NCORES):
        b, p = c // 2, c % 2
        xkvt = np.ascontiguousarray(
            x_full[b].reshape(S // P, P, D)[p::2].reshape(JT * P, D).T
        ).astype(hf)
        in_maps.append(
            {
                "xt": xts[b],
                "xkvt": xkvt,
                "s16": s16,
                "s32": s32,
                "s32m": np.ascontiguousarray(mrows[p]),
            }
        )
    return in_maps


def _get_program():
    if "nc" not in _CACHE:
        _CACHE["nc"] = _build_program()
    return _CACHE["nc"]


def run_on_device(in_maps, trace=False, trace_kwargs=None):
    from concourse import bass_utils

    nc = _get_program()
    return bass_utils.run_bass_kernel_spmd(
        nc,
        in_maps,
        core_ids=list(range(NCORES)),
        trace=trace,
        trace_kwargs=trace_kwargs or {},
    )


def kernel(x, Wq, bq, Wk, bk, Wv, bv):
    x = np.asarray(x, np.float32)
    in_maps = _host_inputs(x, Wq, bq, Wk, bk, Wv, bv)
    res = run_on_device(in_maps)
    out = np.empty((B, S, D + VD), np.float32)
    for b in range(B):
        attT = res.results[2 * b]["att"] + res.results[2 * b + 1]["att"]
        out[b, :, :D] = x[b]
        out[b, :, D:] = attT.T
    return out


# revision 39
# speedup vs baseline: 1.0793x; 1.0793x over previous
"""Trainium2 Bass kernel for nn_AttentionBlock (column-softmax causal attention).

Reference computation (B=4, S=4096, D=128, K=64, V=128):
    Q = x @ Wq.T + bq            [B,S,64]
    Km = x @ Wk.T + bk           [B,S,64]
    Vm = x @ Wv.T + bv           [B,S,128]
    s  = Q @ Km.T / 8            [B,S,S], causal mask j>q -> -1e9
    p  = softmax(s, axis=1)      (softmax over the QUERY axis -- column softmax)
    att = p @ Vm                 [B,S,128]
    out = concat(x, att, dim=2)  [B,S,256]

Algebraic restructure (lets every matmul run bf16 at ~1 cyc/row with full
128-deep contraction):
    s[q,j] = x_q M x_j^T + x_q.a + x_j.b + c   with M = Wq^T Wk / 8,
             a = Wq^T bk / 8, b = Wk^T bq / 8, c = bq.bk / 8.
    The (x_j.b + c) term is constant along the softmax (q) axis for a fixed
    column j, so it CANCELS in softmax(dim=q) and is dropped entirely.
    The x_q.a term folds into G: with G_j = M x_j^T + a (per-partition add),
    s^T[j,q] = sum_d G[d,j] * xT[d,q].
    So: GT = M @ xkv^T + a (tiny matmul), then scores are a single bf16
    128-contraction matmul per (j-tile, q-chunk). No Q/K projections at all.

Flash-style column softmax as in the baseline: E[j,q] = exp(s^T), masked
entries are exp(-1e9)=0; l[j] = sum_q E[j,q] (free-dim reduce);
att^T[v,q] = sum_j (V[j,v]/l[j]) * E[j,q]. Output stays in [v,q] layout --
the HOST transposes (no PE transposes anywhere on device).

Sharding (8 cores): core c -> batch b = c//2, j-tile parity p = c%2.
Host adds the two partial att's per batch.

Pipeline: forward row order (row i = local j-tile i), PV chunk c emitted
after row 2c+1, so the PV matmuls run interleaved with later rows' score
matmuls and the exp stream -- no serial phase C.
"""

import numpy as np

B, S, D = 4, 4096, 128
KD, VD = 64, 128
P = 128
NCORES = 8
JT = 16           # local j-tiles per core
CHUNK = 1536      # score chunk width (PSUM cols, 3 banks)

ROW_W = [S - 2 * i * P for i in range(JT)]          # E row widths
EOFF = [0] * JT
for _i in range(1, JT):
    EOFF[_i] = EOFF[_i - 1] + ROW_W[_i - 1]
ECOLS = EOFF[-1] + ROW_W[-1]                        # 34816

_CACHE = {}


def _build_program():
    from contextlib import ExitStack

    from concourse import bacc, mybir
    from concourse import tile as tile_mod

    dt = mybir.dt
    f32, bf16 = dt.float32, dt.bfloat16
    Alu = mybir.AluOpType
    ActF = mybir.ActivationFunctionType

    nc = bacc.Bacc(
        "TRN2", target_bir_lowering=False, debug=False, num_devices=NCORES
    )

    xt_d = nc.dram_tensor("xt", [P, S], bf16, kind="ExternalInput").ap()
    xkvt_d = nc.dram_tensor("xkvt", [P, JT * P], bf16, kind="ExternalInput").ap()
    # s16: Mt[0:128] | WvT[128:256]
    s16_d = nc.dram_tensor("s16", [P, 256], bf16, kind="ExternalInput").ap()
    # s32: a[0] | bvb[1:129]   s32m: mrow
    s32_d = nc.dram_tensor("s32", [P, 129], f32, kind="ExternalInput").ap()
    s32m_d = nc.dram_tensor("s32m", [P, 256], f32, kind="ExternalInput").ap()
    att_d = nc.dram_tensor("att", [P, S], f32, kind="ExternalOutput").ap()

    with tile_mod.TileContext(nc) as tc, ExitStack() as ctx:
        persist = ctx.enter_context(tc.tile_pool(name="persist", bufs=1))

        xT = persist.tile([P, S], bf16)            # [d, q]
        xkvT = persist.tile([P, JT * P], bf16)     # [d, local j]
        GT = persist.tile([P, JT * P], bf16)       # [d, local j] = M xkv^T + a
        E_all = persist.tile([P, ECOLS], bf16)     # exp(scores^T) rows
        Vp = persist.tile([P, JT, VD], bf16)       # [j, v] scaled by 1/l
        l_all = persist.tile([P, JT], f32)
        linv = persist.tile([P, JT], f32)
        lp2 = persist.tile([P, JT], f32)           # chunk-1 l partials
        lp3 = persist.tile([P, JT], f32)           # chunk-2 l partials
        V_sb = persist.tile([P, JT, VD], f32)      # V + bv, unscaled
        o5a = persist.tile([P, 512], f32)          # PV chunk-5 early partial
        o6a = persist.tile([P, 512], f32)          # PV chunk-6 early partial
        o7a = persist.tile([P, 512], f32)          # PV chunk-7 early partial
        warm = persist.tile([P, 8], f32)           # exp-table warmup scratch
        s16 = persist.tile([P, 256], bf16)
        s32 = persist.tile([P, 129], f32)
        s32m = persist.tile([P, 256], f32)
        Mt = s16[:, 0:128]
        WvT = s16[:, 128:256]
        a_sb = s32[:, 0:1]
        bvb = s32[:, 1:129]
        mrow = s32m

        # ---- input DMAs: ~1.76MB over three queues at ~35-40GB/s each is
        # the startup wall; balance bytes per queue and order every piece
        # by when the pipeline first needs it
        nc.sync.dma_start(out=s16, in_=s16_d)
        nc.sync.dma_start(out=xkvT[:, 0:128], in_=xkvt_d[:, 0:128])
        nc.sync.dma_start(out=xT[:, 0:512], in_=xt_d[:, 0:512])
        nc.sync.dma_start(out=xT[:, 1536:2048], in_=xt_d[:, 1536:2048])
        nc.sync.dma_start(out=xT[:, 2560:3072], in_=xt_d[:, 2560:3072])
        nc.sync.dma_start(out=xkvT[:, 1024:1536], in_=xkvt_d[:, 1024:1536])
        nc.scalar.dma_start(out=s32, in_=s32_d)
        nc.scalar.dma_start(out=xT[:, 512:1024], in_=xt_d[:, 512:1024])
        nc.scalar.dma_start(out=xT[:, 1024:1536], in_=xt_d[:, 1024:1536])
        nc.scalar.dma_start(out=xT[:, 2048:2560], in_=xt_d[:, 2048:2560])
        nc.scalar.dma_start(out=xT[:, 3072:3584], in_=xt_d[:, 3072:3584])
        nc.gpsimd.dma_start(out=s32m, in_=s32m_d)
        nc.gpsimd.dma_start(out=xkvT[:, 128:512], in_=xkvt_d[:, 128:512])
        nc.gpsimd.dma_start(out=xkvT[:, 512:1024], in_=xkvt_d[:, 512:1024])
        nc.gpsimd.dma_start(out=xkvT[:, 1536:2048], in_=xkvt_d[:, 1536:2048])
        nc.gpsimd.dma_start(out=xT[:, 3584:4096], in_=xt_d[:, 3584:4096])

        # load the EXP activation table while DMAs land
        nc.gpsimd.memset(warm, 0.0)
        nc.scalar.activation(
            out=warm, in_=warm, func=ActF.Exp
        )

        with ExitStack() as ph:
            ps = ph.enter_context(
                tc.tile_pool(name="ps", bufs=2, space="PSUM")
            )
            aux = ph.enter_context(
                tc.tile_pool(name="aux", bufs=2, space="PSUM")
            )
            osb = ph.enter_context(tc.tile_pool(name="osb", bufs=2))

            def emit_gt(c0, c1):
                pgt = aux.tile([P, c1 - c0], f32, tag="aux", name=f"gt_{c0}")
                nc.tensor.matmul(
                    pgt,
                    lhsT=Mt,
                    rhs=xkvT[:, c0:c1],
                    start=True,
                    stop=True,
                )
                nc.vector.tensor_scalar(
                    out=GT[:, c0:c1],
                    in0=pgt,
                    scalar1=a_sb,
                    scalar2=None,
                    op0=Alu.add,
                )

            def emit_qk_row(i):
                w = ROW_W[i]
                q0 = 256 * i
                # row 0's first chunk needs only the first xT quarter, so
                # the exp stream starts as soon as that DMA lands
                if i == 0:
                    bounds = [0, 1024, 2560, 4096]
                else:
                    bounds = list(range(0, w, CHUNK)) + [w]
                nch = len(bounds) - 1
                for ci in range(nch):
                    c0, cw = bounds[ci], bounds[ci + 1] - bounds[ci]
                    sc = ps.tile([P, CHUNK], f32, tag="ps", name=f"sc_{i}_{ci}")
                    for s0 in range(0, cw, 512):
                        sw = min(512, cw - s0)
                        off = q0 + c0 + s0
                        nc.tensor.matmul(
                            sc[:, s0 : s0 + sw],
                            lhsT=GT[:, i * P : (i + 1) * P],
                            rhs=xT[:, off : off + sw],
                            start=True,
                            stop=True,
                        )
                        if ci == 0 and s0 == 0:
                            # mask the diagonal block as soon as its slice
                            # lands (overlaps the remaining slice matmuls)
                            nc.vector.tensor_tensor(
                                out=sc[:, : 2 * P],
                                in0=sc[:, : 2 * P],
                                in1=mrow,
                                op=Alu.add,
                            )
                    ecol = EOFF[i] + c0
                    nc.scalar.activation(
                        out=E_all[:, ecol : ecol + cw],
                        in_=sc[:, :cw],
                        func=ActF.Exp,
                        accum_out=[l_all, lp2, lp3][ci][:, i : i + 1],
                    )
                for pp in ([lp2, lp3][: nch - 1]):
                    nc.vector.tensor_tensor(
                        out=l_all[:, i : i + 1],
                        in0=l_all[:, i : i + 1],
                        in1=pp[:, i : i + 1],
                        op=Alu.add,
                    )
                nc.vector.reciprocal(linv[:, i : i + 1], l_all[:, i : i + 1])
                nc.vector.tensor_scalar(
                    out=Vp[:, i, :],
                    in0=V_sb[:, i, :],
                    scalar1=linv[:, i : i + 1],
                    scalar2=None,
                    op0=Alu.mult,
                )

            def emit_v(i):
                # V projection for tile i (runs at startup; needs no l)
                pv = aux.tile([P, VD], f32, tag="aux", name=f"v_{i}")
                nc.tensor.matmul(
                    pv,
                    lhsT=xkvT[:, i * P : (i + 1) * P],
                    rhs=WvT,
                    start=True,
                    stop=True,
                )
                nc.vector.tensor_tensor(
                    out=V_sb[:, i, :], in0=pv, in1=bvb, op=Alu.add
                )

            def emit_pv(c, lo=0, hi=None, merge=None, out_sb=None):
                # att^T chunk c over full-width rows [lo, hi); when hi is
                # None also the half-coverage row 2c+1 closes the group
                tail = hi is None
                hi2 = 2 * c + 1 if tail else hi
                ap = aux.tile([P, 512], f32, tag="aux", name=f"att_{c}_{lo}")
                for ii in range(lo, hi2):
                    ecol = EOFF[ii] + 512 * c - 256 * ii
                    nc.tensor.matmul(
                        ap,
                        lhsT=Vp[:, ii, :],
                        rhs=E_all[:, ecol : ecol + 512],
                        start=(ii == lo),
                        stop=(not tail and ii == hi2 - 1),
                    )
                if tail:
                    i2 = 2 * c + 1
                    nc.tensor.matmul(
                        ap[:, 256:512],
                        lhsT=Vp[:, i2, :],
                        rhs=E_all[:, EOFF[i2] : EOFF[i2] + 256],
                        start=False,
                        stop=True,
                    )
                if out_sb is not None:
                    nc.vector.tensor_copy(out_sb, ap)
                    return
                ob = osb.tile([P, 512], f32, tag="osb", name=f"osb_{c}")
                if merge is None:
                    nc.vector.tensor_copy(ob, ap)
                else:
                    nc.vector.tensor_tensor(
                        out=ob, in0=ap, in1=merge, op=Alu.add
                    )
                nc.sync.dma_start(
                    out=att_d[:, c * 512 : (c + 1) * 512], in_=ob
                )

            emit_gt(0, 128)
            emit_v(0)
            for i in range(JT):
                if i == 1:
                    emit_gt(128, 512)
                    for t in range(1, 4):
                        emit_v(t)
                if i == 3:
                    emit_gt(512, 1024)
                    for t in range(4, 8):
                        emit_v(t)
                if i == 5:
                    emit_gt(1024, 1536)
                    for t in range(8, 12):
                        emit_v(t)
                if i == 7:
                    emit_gt(1536, 2048)
                    for t in range(12, 16):
                        emit_v(t)
                emit_qk_row(i)
                # PV schedule: early partials of the late chunks run as soon
                # as their rows' E lands, so the post-exp tail stays tiny
                if i % 2 == 1 and i <= 9:
                    emit_pv((i - 1) // 2)
                if i == 8:
                    emit_pv(5, lo=0, hi=8, out_sb=o5a)
                if i == 10:
                    emit_pv(6, lo=0, hi=10, out_sb=o6a)
                if i == 11:
                    emit_pv(5, lo=8, merge=o5a)
                    emit_pv(7, lo=0, hi=12, out_sb=o7a)
                if i == 13:
                    emit_pv(6, lo=10, merge=o6a)
            emit_pv(7, lo=12, merge=o7a)

    nc.compile()
    return nc


def _host_inputs(x, Wq, bq, Wk, bk, Wv, bv):
    """Per-core input maps (host does layout prep + tiny precomputes)."""
    import ml_dtypes

    hf = ml_dtypes.bfloat16
    x_full = np.ascontiguousarray(x, dtype=np.float32)
    Wq = np.asarray(Wq, np.float32)
    Wk = np.asarray(Wk, np.float32)
    bk = np.asarray(bk, np.float32)
    Wv = np.asarray(Wv, np.float32)
    bv = np.asarray(bv, np.float32)

    M = (Wq.T @ Wk) / 8.0                      # [D, D]
    Mt = np.ascontiguousarray(M.T).astype(hf)
    a = ((Wq.T @ bk) / 8.0).reshape(D, 1)      # [D, 1]
    WvT = np.ascontiguousarray(Wv.T).astype(hf)
    bvb = np.tile(bv.reshape(1, VD), (P, 1))   # [P, V]

    tri = np.where(
        np.arange(P)[None, :] >= np.arange(P)[:, None], 0.0, -1e9
    ).astype(np.float32)
    mrows = []
    for p in (0, 1):
        m = np.zeros((P, 2 * P), np.float32)
        if p == 0:
            m[:, :P] = tri
        else:
            m[:, :P] = -1e9
            m[:, P:] = tri
        mrows.append(m)

    s16 = np.ascontiguousarray(np.concatenate([Mt, WvT], axis=1))
    s32 = np.ascontiguousarray(
        np.concatenate([a, bvb], axis=1).astype(np.float32)
    )
    xts = [
        np.ascontiguousarray(x_full[b].T.astype(hf)) for b in range(B)
    ]
    in_maps = []
    for c in range(NCORES):
        b, p = c // 2, c % 2
        xkvt = np.ascontiguousarray(
            x_full[b].reshape(S // P, P, D)[p::2].reshape(JT * P, D).T
        ).astype(hf)
        in_maps.append(
            {
                "xt": xts[b],
                "xkvt": xkvt,
                "s16": s16,
                "s32": s32,
                "s32m": np.ascontiguousarray(mrows[p]),
            }
        )
    return in_maps


def _get_program():
    if "nc" not in _CACHE:
        _CACHE["nc"] = _build_program()
    return _CACHE["nc"]


def run_on_device(in_maps, trace=False, trace_kwargs=None):
    from concourse import bass_utils

    nc = _get_program()
    return bass_utils.run_bass_kernel_spmd(
        nc,
        in_maps,
        core_ids=list(range(NCORES)),
        trace=trace,
        trace_kwargs=trace_kwargs or {},
    )


def kernel(x, Wq, bq, Wk, bk, Wv, bv):
    x = np.asarray(x, np.float32)
    in_maps = _host_inputs(x, Wq, bq, Wk, bk, Wv, bv)
    res = run_on_device(in_maps)
    out = np.empty((B, S, D + VD), np.float32)
    for b in range(B):
        attT = res.results[2 * b]["att"] + res.results[2 * b + 1]["att"]
        out[b, :, :D] = x[b]
        out[b, :, D:] = attT.T
    return out
